# revision 36
# baseline (speedup 1.0000x reference)
"""Cached self-attention Trainium2 kernel (v6).

Sharding: 8 cores = 2 batches x 4 head-groups. Core c: batch b=c//4, group
g=c%4 owns heads 4g..4g+3 (columns 512g:512g+512 of the q/k/v projections).

v6 vs v5 (637us):
- softmax-Z finalization entirely off TensorE: GpSimd partition_all_reduce
  replaces the ones-matmul partition reduce, and the reciprocal is taken
  on the broadcast [128,512] result directly (no broadcast matmul, no z
  PSUM bank, no deferred emission machinery).
- bias adds via a precomputed broadcast-bias SBUF tile + DVE add during
  PSUM evacuation (removes the 32 ones-matmul bias adds for v/o).
- psO 4 PSUM banks (was 2: each m-tile stalled ~0.8us on its evac).
- phase-3/v evacuation on DVE explicitly (ScalarE queue blocks on y-DMA
  waits otherwise).
- DMA order: wk, then all 4 x blocks, then wv/wq/cache (x gates k-pass).
- PA 3 PSUM banks. PSUM: pss0(2) pss1(2) PA(3) psq(1) = 8 banks.

Measured v5 facts this builds on: the PE runs at 13/16 throttle (~1.95
GHz) under sustained load, so the matmul stream floor is ~262ns/MM; LDW
overlaps MMs fine; exp on ScalarE (FD=1024) hides under the 5-MM c8
group.
"""
import numpy as np
from contextlib import ExitStack

import concourse.bass as bass
import concourse.tile as tile
from concourse import bacc, mybir
from concourse.bass_isa import ReduceOp
from concourse.bass_utils import run_bass_kernel_spmd

B, S, PC, D, H = 2, 2048, 2048, 2048, 16
HD = D // H            # 128 head dim
GH = H // 4            # 4 heads per core
DG = GH * HD           # 512 head-dims per core
NB = 512               # seq block size
NDC = D // HD          # 16 contraction chunks
NCC = PC // HD         # 16 cache key chunks
NKC = (PC + S) // HD   # 32 total key chunks
F16 = mybir.dt.float16
F32 = mybir.dt.float32
AF = mybir.ActivationFunctionType
ALU = mybir.AluOpType
INV_SQRT_HD = float(1.0 / np.sqrt(HD))

GROUPS = [[0, 1, 2, 3], [4, 5, 6, 7]]


def build():
    nc = bacc.Bacc("TRN2", target_bir_lowering=False, debug=False, num_devices=8)

    def inp(name, shape):
        return nc.dram_tensor(name, shape, F16, kind="ExternalInput").ap()

    xT = inp("xT", [D, S])          # x[b].T
    wq = inp("wq", [D, DG])         # wq[:, 512g:512g+512]
    bq = inp("bq", [DG])            # bq slice / sqrt(HD)
    wk = inp("wk", [D, DG])
    bk = inp("bk", [DG])
    wv = inp("wv", [D, DG])
    bv = inp("bv", [DG])
    ckT = inp("ckT", [DG, PC])      # cache_k[b,:,slice].T
    cv = inp("cv", [PC, DG])        # cache_v[b,:,slice]
    wo = inp("wo", [D, DG])         # wo rows permuted to gather order, cols sliced
    bo = inp("bo", [DG])
    y = nc.dram_tensor("y", [S, DG], F32, kind="ExternalOutput").ap()

    with tile.TileContext(nc) as tc, ExitStack() as ctx:
        res = ctx.enter_context(tc.tile_pool(name="res", bufs=1))
        dram = ctx.enter_context(tc.tile_pool(name="dram", bufs=1, space="DRAM"))

        # tiny whole-kernel residents (issued first: needed by early evacs)
        bq_t = res.tile([HD, GH], F16, tag="bq")
        bk_t = res.tile([HD, GH], F16, tag="bk")
        bv_t = res.tile([1, DG], F16, tag="bv")
        bo_t = res.tile([1, DG], F16, tag="bo")
        ones_r16 = res.tile([1, HD], F16, tag="ones_r16")  # [1,128] ones
        bias_bc = res.tile([HD, DG], F16, tag="bias_bc")   # bv then bo, bcast
        nc.sync.dma_start(bq_t[:], bq.rearrange("(m p) -> p m", p=HD))
        nc.sync.dma_start(bk_t[:], bk.rearrange("(m p) -> p m", p=HD))
        nc.sync.dma_start(bv_t[:], bv[None, :])
        nc.sync.dma_start(bo_t[:], bo[None, :])
        nc.vector.memset(ones_r16[:], 1.0)

        # big residents for phases 1+2
        kTn = res.tile([HD, GH, S], F16, tag="kTn")         # 16KB/p
        vn_t = res.tile([HD, S // HD, DG], F16, tag="vn")   # 16KB/p
        cv_t = res.tile([HD, NCC, DG], F16, tag="cv")       # 16KB/p
        ckT_t = res.tile([HD, GH, PC], F16, tag="ckT")      # 16KB/p

        # attention working pools (whole-kernel scope)
        qp = ctx.enter_context(tc.tile_pool(name="qp", bufs=2))
        ep = ctx.enter_context(tc.tile_pool(name="ep", bufs=3))
        zp = ctx.enter_context(tc.tile_pool(name="zp", bufs=1))
        apool = ctx.enter_context(tc.tile_pool(name="apool", bufs=2))

        # collective bounce buffers, per (head, seq-block)
        bounce_in = [[dram.tile([HD, NB], F16, tag=f"bi{j}_{sb}",
                                name=f"bi{j}_{sb}") for sb in range(4)]
                     for j in range(GH)]
        bounce_out = [[dram.tile([4, HD, NB], F16, tag=f"bg{j}_{sb}",
                                 name=f"bg{j}_{sb}") for sb in range(4)]
                      for j in range(GH)]

        qT_tiles = {}
        lts = {}
        lt_pool = [None]       # set once the phase-3 lt pool is open
        pend = [None]          # deferred normalize+gather for previous block

        def make_fin(j, sb, zsum, PA):
            # reciprocal + normalize + gather for block (j,sb); emitted ~4
            # c8-iterations into the NEXT block so the DVE queue never
            # blocks on the GpSimd all-reduce (which itself can queue
            # behind a peer-skewed collective).
            def go():
                zb = zp.tile([HD, NB], F32, tag="zb")
                nc.vector.reciprocal_approx_fast(zb[:], zsum[:])
                ahead = apool.tile([HD, NB], F16, tag="ah")
                nc.vector.tensor_tensor(ahead[:], PA[:], zb[:], ALU.mult)
                nc.sync.dma_start(bounce_in[j][sb][:], ahead[:])
                nc.gpsimd.collective_compute(
                    "AllGather", ALU.bypass, replica_groups=GROUPS,
                    ins=[bounce_in[j][sb].opt()], outs=[bounce_out[j][sb].opt()])
                if lt_pool[0] is not None:
                    lt = lt_pool[0].tile([HD, 4, NB], F16, tag=f"lt{j}_{sb}",
                                         name=f"lt{j}_{sb}")
                    nc.sync.dma_start(
                        lt[:], bounce_out[j][sb].rearrange("r p n -> p r n"))
                    lts[(j, sb)] = lt
            return go

        def emit_pending():
            if pend[0] is not None:
                pend[0]()
                pend[0] = None

        def make_qproj(jq, pool, wqt, xres):
            qt = qp.tile([HD, S], F16, tag="qT", name=f"qT{jq}")
            qT_tiles[jq] = qt
            cur = {}

            def step(s, gate=None):
                sbq, kc = divmod(s, NDC)
                if kc == 0:
                    cur["psq"] = pool.tile([HD, NB], F32, tag="psq", name="psq")
                psq = cur["psq"]
                wchunk = wqt[:, kc, HD * jq:HD * (jq + 1)]
                if gate is not None and kc % 4 == 0:
                    # Pace this filler matmul with the attention pipeline:
                    # route its lhsT through a DVE op that (exactly)
                    # adds 0*gate, so the Tile scheduler cannot hoist the
                    # whole projection ahead of the exp stream (which
                    # would burn all the filler at once and then starve).
                    zc = zp.tile([HD, HD], F16, tag="zc")
                    nc.vector.tensor_scalar_mul(zc[:], gate[:, 0, 0:HD], 0.0)
                    ws = zp.tile([HD, HD], F16, tag="ws", bufs=2)
                    nc.vector.tensor_tensor(ws[:], wchunk, zc[:], ALU.add)
                    wchunk = ws[:]
                nc.tensor.matmul(psq[:], wchunk,
                                 xres[:, kc, NB * sbq:NB * (sbq + 1)],
                                 start=(kc == 0), stop=(kc == NDC - 1),
                                 skip_group_check=True)
                if kc == NDC - 1:
                    nc.scalar.activation(qt[:, NB * sbq:NB * (sbq + 1)], psq[:],
                                         AF.Identity, bias=bq_t[:, jq:jq + 1],
                                         scale=INV_SQRT_HD)
            return step

        def make_p3fill(pB, wot):
            # m-tile 0 of the output projection, interleaved into head 3's
            # attention (which is ScalarE-paced, so these fill TensorE idle
            # slivers); shrinks serial phase 3 by one m-tile.
            st = {}

            def fill(s):
                if s > 16:
                    return
                if s == 0:
                    st["psO"] = pB.tile([HD, NB], F32, tag="psO", name="psOf")
                if s < 16:
                    jf, rf = divmod(s, 4)
                    nc.tensor.matmul(st["psO"][:], lts[(jf, 0)][:, rf, 0:HD],
                                     wot[:, 4 * jf + rf, :],
                                     start=(s == 0), stop=(s == 15),
                                     skip_group_check=True)
                else:
                    otf = zp.tile([HD, NB], F32, tag="otf")
                    nc.vector.tensor_tensor(otf[:], st["psO"][:], bias_bc[:],
                                            ALU.add)
                    nc.sync.dma_start(y[0:HD, :], otf[:])
            return fill

        def att_head(pB, j, qnext, p3fill=None):
            scope = nc.named_scope(f"h{j}")
            scope.__enter__()
            for sb in range(4):
                PA = pB.tile([HD, NB], F32, tag=f"PA{sb % 2}", name="PA")
                zacc2 = zp.tile([HD, 2, NB], F16, tag="za")
                qTs = qT_tiles[j][:, NB * sb:NB * (sb + 1)]
                for c8 in range(NKC // 2):
                    pss = pB.tile([HD, 2, NB], F32, tag=f"pss{c8 % 2}",
                                  name="pss")
                    for i in range(2):
                        c = 2 * c8 + i
                        if c < NCC:
                            kt = ckT_t[:, j, HD * c:HD * (c + 1)]
                        else:
                            kt = kTn[:, j, HD * (c - NCC):HD * (c - NCC + 1)]
                        nc.tensor.matmul(pss[:, i, :], kt, qTs,
                                         start=True, stop=True)
                    e2 = ep.tile([HD, 2, NB], F16, tag="e2")
                    nc.scalar.activation(e2[:], pss[:], AF.Exp)
                    for i in range(2):
                        c = 2 * c8 + i
                        if c < NCC:
                            vt = cv_t[:, c, HD * j:HD * (j + 1)]
                        else:
                            vt = vn_t[:, c - NCC, HD * j:HD * (j + 1)]
                        nc.tensor.matmul(PA[:], vt, e2[:, i, :],
                                         start=(c == 0), stop=(c == NKC - 1),
                                         skip_group_check=True)
                    if c8 == 0:
                        nc.vector.tensor_copy(zacc2[:], e2[:])
                    else:
                        nc.vector.tensor_tensor(zacc2[:], zacc2[:], e2[:],
                                                ALU.add)
                    if c8 == 4:
                        emit_pending()
                    if qnext is not None:
                        qnext(16 * sb + c8)
                    if p3fill is not None and sb == 3:
                        p3fill(c8)
                # Z partition reduce: DVE fold + GpSimd all-reduce, inline;
                # the consumer chain is deferred into the next block.
                zfold = zp.tile([HD, NB], F16, tag="zf")
                nc.vector.tensor_tensor(zfold[:], zacc2[:, 0, :],
                                        zacc2[:, 1, :], ALU.add)
                zsum = zp.tile([HD, NB], F32, tag="zs")
                nc.gpsimd.partition_all_reduce(zsum[:], zfold[:], HD,
                                               ReduceOp.add)
                pend[0] = make_fin(j, sb, zsum, PA)
            scope.__exit__(None, None, None)

        with ExitStack() as cx:
            # ---- x-resident era: projections + attention heads 0-2 ----
            px = cx.enter_context(tc.tile_pool(name="px", bufs=1))
            wkt = px.tile([HD, NDC, DG], F16, tag="wkt")    # 16KB/p
            xres = px.tile([HD, NDC, S], F16, tag="xres")   # 64KB/p
            wvt = px.tile([HD, NDC, DG], F16, tag="wvt")
            wqt = px.tile([HD, NDC, DG], F16, tag="wqt")

            # DMA order tuned so compute starts ASAP.
            xr = xT.rearrange("(kc p) s -> p kc s", p=HD)
            wkr = wk.rearrange("(kc p) n -> p kc n", p=HD)
            nc.sync.dma_start(wkt[:, :, 0:HD], wkr[:, :, 0:HD])
            nc.sync.dma_start(xres[:, :, 0:NB], xr[:, :, 0:NB])
            nc.sync.dma_start(wkt[:, :, HD:2 * HD], wkr[:, :, HD:2 * HD])
            nc.sync.dma_start(wkt[:, :, 2 * HD:DG], wkr[:, :, 2 * HD:DG])
            nc.sync.dma_start(xres[:, :, NB:2 * NB], xr[:, :, NB:2 * NB])
            for sb in range(2, 4):
                nc.sync.dma_start(xres[:, :, NB * sb:NB * (sb + 1)],
                                  xr[:, :, NB * sb:NB * (sb + 1)])
            nc.sync.dma_start(wvt[:], wv.rearrange("(kc p) n -> p kc n", p=HD))
            nc.sync.dma_start(wqt[:], wq.rearrange("(kc p) n -> p kc n", p=HD))
            nc.sync.dma_start(cv_t[:], cv.rearrange("(ss p) d -> p ss d", p=HD))
            nc.sync.dma_start(ckT_t[:], ckT.rearrange("(m p) s -> p m s", p=HD))

            with tc.tile_pool(name="pA", bufs=1, space="PSUM") as pA:
                # broadcast bv across partitions once (zero-cost bias adds on
                # the v evacuations)
                psb0 = pA.tile([HD, DG], F32, tag="psv0", name="psb0")
                nc.tensor.matmul(psb0[:], ones_r16[:], bv_t[:],
                                 start=True, stop=True)
                nc.vector.tensor_copy(bias_bc[:], psb0[:])

                # ---- k-pass (seq-block-major, streams behind the x DMA) ----
                kscope = nc.named_scope("kp")
                kscope.__enter__()
                for sb in range(4):
                    for m in range(GH):
                        psk = pA.tile([HD, NB], F32, tag=f"kq{m % 2}",
                                      name="psk")
                        for kc in range(NDC):
                            nc.tensor.matmul(
                                psk[:], wkt[:, kc, HD * m:HD * (m + 1)],
                                xres[:, kc, NB * sb:NB * (sb + 1)],
                                start=(kc == 0), stop=(kc == NDC - 1))
                        nc.scalar.activation(kTn[:, m, NB * sb:NB * (sb + 1)],
                                             psk[:], AF.Identity,
                                             bias=bk_t[:, m:m + 1])
                kscope.__exit__(None, None, None)

                # ---- v-pass with head-0 q projection interleaved ----
                vscope = nc.named_scope("vp")
                vscope.__enter__()
                q0 = make_qproj(0, pA, wqt, xres)
                qi = 0
                for ss in range(S // HD):
                    psv = pA.tile([HD, DG], F32, tag=f"psv{ss % 2}", name="psv")
                    for kc in range(NDC):
                        nc.tensor.matmul(psv[:],
                                         xres[:, kc, HD * ss:HD * (ss + 1)],
                                         wvt[:, kc, :],
                                         start=(kc == 0),
                                         stop=(kc == NDC - 1))
                    nc.vector.tensor_tensor(vn_t[:, ss, :], psv[:], bias_bc[:],
                                            ALU.add)
                    for _ in range(4):
                        q0(qi)
                        qi += 1
                # rewrite bias_bc with broadcast bo for phase 3
                psb1 = pA.tile([HD, DG], F32, tag="psv1", name="psb1")
                nc.tensor.matmul(psb1[:], ones_r16[:], bo_t[:],
                                 start=True, stop=True)
                nc.vector.tensor_copy(bias_bc[:], psb1[:])
                vscope.__exit__(None, None, None)

            # ---- attention heads 0-2, next head's q proj interleaved ----
            pBs = ExitStack()
            pB = pBs.enter_context(tc.tile_pool(name="pB", bufs=1, space="PSUM"))
            att_head(pB, 0, make_qproj(1, pB, wqt, xres))
            att_head(pB, 1, make_qproj(2, pB, wqt, xres))
            att_head(pB, 2, make_qproj(3, pB, wqt, xres))

        # ---- x freed; load phase-3 operands, run head 3, then out-proj ----
        ltp = ctx.enter_context(tc.tile_pool(name="ltp", bufs=1))
        wop = ctx.enter_context(tc.tile_pool(name="wop", bufs=1))
        lt_pool[0] = ltp
        wot = wop.tile([HD, 16, NB], F16, tag="wo")
        nc.sync.dma_start(wot[:], wo.rearrange("(c p) n -> p c n", p=HD))
        # sb-major so the (j,0) tiles the head-3 filler needs arrive first
        for sb in range(4):
            for j in range(GH - 1):
                if pend[0] is not None and (j, sb) == (GH - 2, 3):
                    # (2,3)'s gather is still pending (deferred into head
                    # 3's stream); its lt load is emitted there too.
                    continue
                lt = ltp.tile([HD, 4, NB], F16, tag=f"lt{j}_{sb}",
                              name=f"lt{j}_{sb}")
                nc.sync.dma_start(lt[:],
                                  bounce_out[j][sb].rearrange("r p n -> p r n"))
                lts[(j, sb)] = lt

        p3fill = make_p3fill(pB, wot)
        att_head(pB, 3, None, p3fill=p3fill)
        p3fill(16)              # evacuate + store filler m-tile 0
        emit_pending()          # normalize + gather for (3,3)
        pBs.close()

        # ---- phase 3: output projection ----
        p3scope = nc.named_scope("p3")
        p3scope.__enter__()
        with tc.tile_pool(name="pC", bufs=1, space="PSUM") as pC, \
             tc.tile_pool(name="p3", bufs=4) as p3:
            for m in range(1, S // HD):
                sb3, o = divmod(m, 4)
                psO = pC.tile([HD, NB], F32, tag=f"psO{m % 4}", name="psO")
                for j in range(GH):
                    for r in range(4):
                        nc.tensor.matmul(
                            psO[:], lts[(j, sb3)][:, r, HD * o:HD * (o + 1)],
                            wot[:, 4 * j + r, :],
                            start=(j == 0 and r == 0),
                            stop=(j == GH - 1 and r == 3),
                            skip_group_check=True)
                ot = p3.tile([HD, NB], F32, tag="ot")
                nc.vector.tensor_tensor(ot[:], psO[:], bias_bc[:], ALU.add)
                nc.sync.dma_start(y[HD * m:HD * (m + 1), :], ot[:])
        p3scope.__exit__(None, None, None)

    nc.compile()
    return nc


_BUILT = None


def get_built():
    global _BUILT
    if _BUILT is None:
        _BUILT = build()
    return _BUILT


def make_in_maps(x, cache_k, cache_v, wq, bq, wk, bk, wv, bv, wo, bo):
    x = np.asarray(x)
    cache_k = np.asarray(cache_k)
    cache_v = np.asarray(cache_v)
    wq, bq = np.asarray(wq), np.asarray(bq)
    wk, bk = np.asarray(wk), np.asarray(bk)
    wv, bv = np.asarray(wv), np.asarray(bv)
    wo, bo = np.asarray(wo), np.asarray(bo)

    # permute wo rows to match gather order: lhsT chunk jr=(4j+r) holds head 4r+j
    perm = np.concatenate([
        np.arange(HD * (4 * r + j), HD * (4 * r + j) + HD)
        for j in range(GH) for r in range(4)
    ])
    wo_p = wo[perm, :]

    in_maps = []
    for c in range(8):
        b, g = divmod(c, 4)
        sl = slice(DG * g, DG * (g + 1))
        in_maps.append({
            "xT": np.ascontiguousarray(x[b].T).astype(np.float16),
            "wq": wq[:, sl].astype(np.float16),
            "bq": (bq[sl] * INV_SQRT_HD).astype(np.float16),
            "wk": wk[:, sl].astype(np.float16),
            "bk": bk[sl].astype(np.float16),
            "wv": wv[:, sl].astype(np.float16),
            "bv": bv[sl].astype(np.float16),
            "ckT": np.ascontiguousarray(cache_k[b][:, sl].T).astype(np.float16),
            "cv": cache_v[b][:, sl].astype(np.float16),
            "wo": wo_p[:, sl].astype(np.float16),
            "bo": bo[sl].astype(np.float16),
        })
    return in_maps


def assemble(results):
    out = np.empty((B, S, D), np.float32)
    for c in range(8):
        b, g = divmod(c, 4)
        out[b, :, DG * g:DG * (g + 1)] = results[c]["y"]
    return out


def kernel(**inputs):
    nc = get_built()
    in_maps = make_in_maps(**inputs)
    res = run_bass_kernel_spmd(nc, in_maps, core_ids=list(range(8)))
    return assemble(res.results)
